# revision 25
# baseline (speedup 1.0000x reference)
"""Trainium2 Bass kernel for ContextHyperLinearSSM.

Computes out[b,:] = x[b,:] @ (WA[context[b]] * adj_xx) + u[b,:] @ (WB[context[b]] * adj_xu)

Strategy: shard the CONTEXT axis across the 8 cores (64 contexts each).
The host groups samples by context, masks the weight banks with the
adjacency masks, and quantizes the weights to fp8-e3m4 (x2^6 scale; the
inverse 2^-6 is folded into the bf16 activations — both scalings are exact
powers of two, so the only quantization error is the e3m4 weight rounding,
measured at 1.3e-2 absmax/scale against the fp32 reference).  Each core
streams its 64 contexts' weights from HBM exactly once and runs 3
accumulating mixed-dtype matmuls (bf16 stationary x fp8 moving) per
context.  Each sample's row is computed by exactly one core, so the
host-side unshard is a pure scatter.

Contexts are globally sorted by sample count and dealt round-robin to the
cores, so every core sees the same per-group padded size G_g (required:
one SPMD program serves all cores) and the padding tracks the count
distribution.  All activations ship in one prefetched DMA; weights stream
one merged DMA per group of CT contexts, groups alternating between the
two HWDGE rings (sync/scalar) so descriptor-generation cost and
completion-semaphore reuse never serialize the stream.  Group 0 is split
into per-half DMAs on both rings (shorter pipeline-fill latency) and the
final, smallest group into per-context DMAs (matmuls start as soon as
their slice of the last transfer lands).

Compute: all CT contexts of a group accumulate into ONE PSUM bank packed
as 4 x 32-aligned partition slots x 2 free halves; matmuls are emitted in
k-major waves cycling the four 128x32 column tiles of the PE array so
four matmuls execute concurrently.  One full-width copy per group (vector
and scalar engines alternating) drains PSUM into a shared bf16 staging
tile, which is flushed by a few consolidated multi-group DMAs.
"""

import sys

sys.path.insert(0, "/opt/trn_rl_repo")

import ml_dtypes
import numpy as np

import concourse.bass as bass
import concourse.mybir as mybir
import concourse.tile as tile
from concourse import bacc
from concourse.bass_utils import run_bass_kernel_spmd

N_CORES = 8
CT = 8  # contexts per PSUM group
WSCALE = 64.0  # 2^6: weights *= WSCALE (into e3m4 range), acts /= WSCALE

BF16 = ml_dtypes.bfloat16
FP8 = ml_dtypes.float8_e3m4


def _install_profile_shim():
    """Register the NTFF profile hook that trn_boot skips when
    antenv.axon_hooks is missing from the image (profiling only)."""
    import types
    if "antenv.axon_hooks" in sys.modules:
        return
    try:
        from trn_agent_boot.trn_boot import _ntff_profile_via_ctypes
        hook = _ntff_profile_via_ctypes("/opt/axon/libaxon_pjrt.so")
    except Exception:
        hook = None
    mod = types.ModuleType("antenv.axon_hooks")
    mod.get_axon_ntff_profile_hook = lambda: hook
    mod.set_axon_ntff_profile_hook = lambda h: None
    sys.modules["antenv.axon_hooks"] = mod


def _geometry(S, A, Gs):
    """Static geometry shared by host and device.

    PSUM packing: context c of a group -> bank t=c//CPT, partition slot
    sl=(c%CPT)%NSL (SLP-aligned), free half cf=(c%CPT)//NSL.
    """
    HS = S // 128
    K = HS + 1
    CH = CT // 2
    WF = CH * K * S
    FF = max(1, min(CT, 512 // S))
    SLP = 32 if max(Gs) <= 32 else 64
    NSL = 128 // SLP
    CPT = min(CT, NSL * FF)
    T = -(-CT // CPT)
    assert T * CPT == CT
    NG = len(Gs)
    # out-staging splits: [0,NG/2), [NG/2,NG-1), [NG-1,NG) — early flushes
    # plus a tiny final one (groups are sorted largest-first)
    splits = [(0, NG // 2), (NG // 2, NG - 1), (NG - 1, NG)]
    splits = [(a, b) for a, b in splits if b > a]
    off_a = 0
    offs_a = []
    for G in Gs:
        offs_a.append(off_a)
        off_a += 2 * CH * K * G
    OW = T * FF * S  # staging elems per partition line per group
    # early splits flush row-limited (GH rows per slot); the final split
    # is one full-partition DMA (one issue on the latency-critical tail)
    off_o = 0
    offs_o = []
    for si, (a, b) in enumerate(splits):
        offs_o.append(off_o)
        if si < len(splits) - 1:
            off_o += NSL * max(Gs[a:b]) * (b - a) * OW
        else:
            off_o += 128 * (b - a) * OW
    return dict(HS=HS, K=K, CH=CH, WF=WF, FF=FF, SLP=SLP, NSL=NSL,
                CPT=CPT, T=T, NG=NG, splits=splits, offs_a=offs_a,
                AL=off_a, OW=OW, offs_o=offs_o, OL=off_o)


def _build_program(S, A, Gs):
    """Build the per-core Bass program for per-group sizes Gs."""
    f32 = mybir.dt.float32
    bf16 = mybir.dt.bfloat16
    fp8 = mybir.dt.float8e3
    nc = bacc.Bacc("TRN2", target_bir_lowering=False)

    geo = _geometry(S, A, Gs)
    K, CH, WF, FF = geo["K"], geo["CH"], geo["WF"], geo["FF"]
    SLP, NSL, CPT, T = geo["SLP"], geo["NSL"], geo["CPT"], geo["T"]
    NG, OW = geo["NG"], geo["OW"]
    assert S % 128 == 0 and A == 128

    wts = nc.dram_tensor("wts", [NG, 128, 2 * WF], fp8,
                         kind="ExternalInput").ap()
    acts = nc.dram_tensor("acts", [128, geo["AL"]], bf16,
                          kind="ExternalInput").ap()
    out = nc.dram_tensor("out", [geo["OL"]], bf16,
                         kind="ExternalOutput").ap()

    with tile.TileContext(nc) as tc:
        with (
            tc.tile_pool(name="a", bufs=1) as apool,
            tc.tile_pool(name="w", bufs=NG) as wpool,
            tc.tile_pool(name="o", bufs=1) as opool,
            tc.tile_pool(name="psum", bufs=8, space="PSUM") as psum,
        ):
            # all activations prefetched in one DMA (small: ~4KB/partition)
            at = apool.tile([128, geo["AL"]], bf16)
            nc.scalar.dma_start(at[:], acts[:])
            # shared output staging tile, flushed by consolidated DMAs
            os_t = opool.tile([128, NG * OW], bf16)

            for g in range(NG):
                G = Gs[g]
                ring = nc.sync if g % 2 == 0 else nc.scalar
                wt = wpool.tile([128, 2 * WF], fp8, tag="wt", name=f"wt_{g}")
                if g == 0:
                    # pipeline fill: one half per ring, in parallel
                    nc.sync.dma_start(wt[:, :WF], wts[0, :, :WF])
                    nc.scalar.dma_start(wt[:, WF:], wts[0, :, WF:])
                elif g == NG - 1:
                    # tail group: per-context weight DMAs (alternating
                    # rings) so the last matmuls start as soon as their
                    # slice lands and completion receipts overlap
                    for hf in range(2):
                        for ci in range(CH):
                            lo = hf * WF + ci * K * S
                            r = nc.sync if (hf * CH + ci) % 2 == 0 \
                                else nc.scalar
                            r.dma_start(wt[:, lo:lo + K * S],
                                        wts[g, :, lo:lo + K * S])
                else:
                    ring.dma_start(wt[:], wts[g])

                def views(c):
                    hf, ci = divmod(c, CH)
                    wv = wt[:, hf * WF + ci * K * S:
                            hf * WF + (ci + 1) * K * S] \
                        .rearrange("p (k s) -> p k s", k=K)
                    a0 = geo["offs_a"][g] + (hf * CH + ci) * K * G
                    av = at[:, a0:a0 + K * G] \
                        .rearrange("p (k g) -> p k g", k=K)
                    return wv, av

                ps_tiles = [psum.tile([128, FF * S], f32, tag="ps",
                                      name=f"ps_{g}_{t}")
                            for t in range(T)]
                # consecutive contexts cycle the NSL column tiles of the
                # PE array so their streams execute concurrently; each
                # context's K accumulating matmuls stay adjacent (the
                # LDWEIGHTS lookahead only tracks one shadow per tile)
                for c in range(CT):
                    wv, av = views(c)
                    t, r2 = divmod(c, CPT)
                    sl, cf = r2 % NSL, r2 // NSL
                    pslice = ps_tiles[t][sl * SLP:sl * SLP + G,
                                         cf * S:cf * S + S]
                    for k in range(K):
                        nc.tensor.matmul(
                            pslice,
                            lhsT=av[:, k, :],
                            rhs=wv[:, k, :],
                            start=(k == 0), stop=(k == K - 1),
                            tile_position=(0, sl * SLP))
                o0 = g * T * FF * S
                if g == NG - 1:
                    # tail group: split the drain across vector+scalar so
                    # the latency-critical final chain halves
                    for t in range(T):
                        half = FF * S // 2
                        nc.vector.tensor_copy(
                            os_t[:, o0 + t * FF * S:
                                 o0 + t * FF * S + half],
                            ps_tiles[t][:, :half])
                        nc.scalar.copy(
                            os_t[:, o0 + t * FF * S + half:
                                 o0 + (t + 1) * FF * S],
                            ps_tiles[t][:, half:])
                else:
                    for t in range(T):
                        eng = (nc.vector.tensor_copy if g % 2 == 0
                               else nc.scalar.copy)
                        eng(os_t[:, o0 + t * FF * S:
                                 o0 + (t + 1) * FF * S],
                            ps_tiles[t][:, :])

            # output flushes: early splits row-limited per partition slot
            # (mid-stream, issue cost is hidden); the final split is one
            # full-partition DMA so the tail pays a single issue
            for si, (a, b) in enumerate(geo["splits"]):
                W = (b - a) * OW
                if si < len(geo["splits"]) - 1:
                    GH = max(Gs[a:b])
                    for sl in range(NSL):
                        dst = out[geo["offs_o"][si] + sl * GH * W:
                                  geo["offs_o"][si] + (sl + 1) * GH * W]
                        ring = (nc.sync if (si * NSL + sl) % 2 == 0
                                else nc.scalar)
                        ring.dma_start(
                            dst.rearrange("(gh w) -> gh w", gh=GH),
                            os_t[sl * SLP:sl * SLP + GH, a * OW:b * OW])
                else:
                    dst = out[geo["offs_o"][si]:geo["offs_o"][si] + 128 * W]
                    nc.sync.dma_start(
                        dst.rearrange("(p w) -> p w", p=128),
                        os_t[:, a * OW:b * OW])

    nc.compile()
    return nc


def kernel(x, u, WA, WB, adj_xx, adj_xu, context, _trace=False):
    B, S = x.shape
    _, A = u.shape
    C = WA.shape[0]
    assert C % N_CORES == 0
    CP = C // N_CORES
    assert CP % CT == 0
    NG = CP // CT

    # ---- host-side shard: count-sorted contexts, dealt round-robin ----
    context = np.asarray(context)
    cnt = np.bincount(context, minlength=C)
    perm = np.argsort(-cnt, kind="stable")          # contexts by count desc
    # context at global rank r -> core r%8, position r//8; group = pos//CT.
    # All cores share one program, so G_g is set by the chunk's global max
    # count = count at rank g*CT*N_CORES.
    Gs = []
    for g in range(NG):
        m = int(cnt[perm[g * CT * N_CORES]])
        Gs.append(max(2, ((m + 1) // 2) * 2))

    geo = _geometry(S, A, Gs)
    HS, K, CH, WF = geo["HS"], geo["K"], geo["CH"], geo["WF"]
    FF, SLP, NSL, CPT, T, OW = (geo["FF"], geo["SLP"], geo["NSL"],
                                geo["CPT"], geo["T"], geo["OW"])

    order = np.argsort(context, kind="stable")
    starts = np.zeros(C + 1, np.int64)
    starts[1:] = np.cumsum(cnt)

    def group_rows(ctx_ids, G):
        """gidx [len,G] sample indices (clamped) + valid mask."""
        j = np.arange(G)
        cc = cnt[ctx_ids][:, None]
        valid = j[None, :] < cc
        pos = starts[ctx_ids][:, None] + np.minimum(j[None, :],
                                                    np.maximum(cc - 1, 0))
        return order[pos], valid

    inv = np.float32(1.0 / WSCALE)
    x = np.asarray(x, np.float32) * inv
    u = np.asarray(u, np.float32) * inv

    # pre-mask the weight banks, scale into e3m4 range, quantize on host
    Am = (np.asarray(WA, np.float32) * np.float32(WSCALE)
          * np.asarray(adj_xx, np.float32)).astype(FP8)    # [C, S, S]
    Bm = (np.asarray(WB, np.float32) * np.float32(WSCALE)
          * np.asarray(adj_xu, np.float32)).astype(FP8)    # [C, A, S]

    in_maps = []
    scatter = []   # per core: list of (ctx_ids, gidx, valid) per group
    for k in range(N_CORES):
        wblob = np.empty((NG, 128, 2, CH, K, S), FP8)
        ablob = np.zeros((128, geo["AL"]), BF16)
        sc = []
        for g in range(NG):
            G = Gs[g]
            ctx_ids = perm[(g * CT + np.arange(CT)) * N_CORES + k]
            gidx, valid = group_rows(ctx_ids, G)           # [CT, G]
            sc.append((ctx_ids, gidx, valid))
            XpT = x[gidx].transpose(0, 2, 1).astype(BF16)  # [CT, S, G]
            UpT = u[gidx].transpose(0, 2, 1).astype(BF16)  # [CT, A, G]
            wb = wblob[g].transpose(1, 2, 0, 3, 4)         # [2,CH,128,K,S]
            wb[..., 0, :] = Bm[ctx_ids].reshape(2, CH, 128, S)
            wb[..., 1:, :] = Am[ctx_ids].reshape(2, CH, HS, 128, S) \
                .transpose(0, 1, 3, 2, 4)
            A3 = ablob[:, geo["offs_a"][g]:
                       geo["offs_a"][g] + 2 * CH * K * G] \
                .reshape(128, 2, CH, K, G).transpose(1, 2, 3, 0, 4)
            A3[:, :, 0] = UpT.reshape(2, CH, 128, G)
            A3[:, :, 1:] = XpT.reshape(2, CH, HS, 128, G)
        in_maps.append({"wts": wblob.reshape(NG, 128, 2 * WF),
                        "acts": ablob})
        scatter.append(sc)

    if _trace:
        _install_profile_shim()
    nc = _build_program(S, A, Gs)
    res = run_bass_kernel_spmd(nc, in_maps, core_ids=list(range(N_CORES)),
                               trace=_trace)

    # unscatter: early splits are [NSL, GH, b-a, T, FF, S]; final split
    # is [128, b-a, T, FF, S]
    out_full = np.zeros((B, S), np.float32)
    for k, r in enumerate(res.results):
        v = np.asarray(r["out"]).astype(np.float32)
        for si, (a, b) in enumerate(geo["splits"]):
            last = si == len(geo["splits"]) - 1
            if last:
                blk = v[geo["offs_o"][si]:geo["offs_o"][si]
                        + 128 * (b - a) * OW] \
                    .reshape(128, b - a, T, FF, S)
            else:
                GH = max(Gs[a:b])
                blk = v[geo["offs_o"][si]:geo["offs_o"][si]
                        + NSL * GH * (b - a) * OW] \
                    .reshape(NSL, GH, b - a, T, FF, S)
            for g in range(a, b):
                ctx_ids, gidx, valid = scatter[k][g]
                for c in range(CT):
                    t, r2 = divmod(c, CPT)
                    sl, cf = r2 % NSL, r2 // NSL
                    if last:
                        rows = blk[sl * SLP:sl * SLP + Gs[g],
                                   g - a, t, cf, :]          # [G, S]
                    else:
                        rows = blk[sl, :Gs[g], g - a, t, cf, :]
                    m = valid[c]
                    out_full[gidx[c][m]] = rows[m]

    if _trace:
        return out_full, res

    return out_full


# revision 29
# speedup vs baseline: 1.0833x; 1.0833x over previous
"""Trainium2 Bass kernel for ContextHyperLinearSSM.

Computes out[b,:] = x[b,:] @ (WA[context[b]] * adj_xx) + u[b,:] @ (WB[context[b]] * adj_xu)

Strategy: shard the CONTEXT axis across the 8 cores (64 contexts each).
The host groups samples by context, masks the weight banks with the
adjacency masks, and quantizes the weights to fp8-e3m4 (x2^6 scale; the
inverse 2^-6 is folded into the bf16 activations — both scalings are exact
powers of two, so the only quantization error is the e3m4 weight rounding,
measured at 1.3e-2 absmax/scale against the fp32 reference).  Each core
streams its 64 contexts' weights from HBM exactly once and runs 3
accumulating mixed-dtype matmuls (bf16 stationary x fp8 moving) per
context.  Each sample's row is computed by exactly one core, so the
host-side unshard is a pure scatter.

Contexts are globally sorted by sample count and dealt round-robin to the
cores, so every core sees the same per-group padded size G_g (required:
one SPMD program serves all cores) and the padding tracks the count
distribution.  All activations ship in one prefetched DMA; weights stream
one merged DMA per group of CT contexts, groups alternating between the
two HWDGE rings (sync/scalar) so descriptor-generation cost and
completion-semaphore reuse never serialize the stream.  Group 0 is split
into per-half DMAs on both rings (shorter pipeline-fill latency) and the
final, smallest group into per-context DMAs (matmuls start as soon as
their slice of the last transfer lands).

Compute: all CT contexts of a group accumulate into ONE PSUM bank packed
as 4 x 32-aligned partition slots x 2 free halves; matmuls are emitted in
k-major waves cycling the four 128x32 column tiles of the PE array so
four matmuls execute concurrently.  One full-width copy per group (vector
and scalar engines alternating) drains PSUM into a shared bf16 staging
tile, which is flushed by a few consolidated multi-group DMAs.
"""

import sys

sys.path.insert(0, "/opt/trn_rl_repo")

import ml_dtypes
import numpy as np

import concourse.bass as bass
import concourse.mybir as mybir
import concourse.tile as tile
from concourse import bacc
from concourse.bass_utils import run_bass_kernel_spmd

N_CORES = 8
CT = 8  # contexts per PSUM group
WSCALE = 64.0  # 2^6: weights *= WSCALE (into e3m4 range), acts /= WSCALE

BF16 = ml_dtypes.bfloat16
FP8 = ml_dtypes.float8_e3m4


def _install_profile_shim():
    """Register the NTFF profile hook that trn_boot skips when
    antenv.axon_hooks is missing from the image (profiling only)."""
    import types
    if "antenv.axon_hooks" in sys.modules:
        return
    try:
        from trn_agent_boot.trn_boot import _ntff_profile_via_ctypes
        hook = _ntff_profile_via_ctypes("/opt/axon/libaxon_pjrt.so")
    except Exception:
        hook = None
    mod = types.ModuleType("antenv.axon_hooks")
    mod.get_axon_ntff_profile_hook = lambda: hook
    mod.set_axon_ntff_profile_hook = lambda h: None
    sys.modules["antenv.axon_hooks"] = mod


def _geometry(S, A, Gs):
    """Static geometry shared by host and device.

    PSUM packing: context c of a group -> bank t=c//CPT, partition slot
    sl=(c%CPT)%NSL (SLP-aligned), free half cf=(c%CPT)//NSL.
    """
    HS = S // 128
    K = HS + 1
    CH = CT // 2
    WF = CH * K * S
    FF = max(1, min(CT, 512 // S))
    SLP = 32 if max(Gs) <= 32 else 64
    NSL = 128 // SLP
    CPT = min(CT, NSL * FF)
    T = -(-CT // CPT)
    assert T * CPT == CT
    NG = len(Gs)
    # out-staging splits: [0,NG/2), [NG/2,NG-1), [NG-1,NG) — early flushes
    # plus a tiny final one (groups are sorted largest-first)
    splits = [(0, NG // 2), (NG // 2, NG - 1), (NG - 1, NG)]
    splits = [(a, b) for a, b in splits if b > a]
    off_a = 0
    offs_a = []
    for G in Gs:
        offs_a.append(off_a)
        off_a += 2 * CH * K * G
    OW = T * FF * S  # staging elems per partition line per group
    return dict(HS=HS, K=K, CH=CH, WF=WF, FF=FF, SLP=SLP, NSL=NSL,
                CPT=CPT, T=T, NG=NG, splits=splits, offs_a=offs_a,
                AL=off_a, OW=OW, OL=128 * NG * OW)


def _build_program(S, A, Gs):
    """Build the per-core Bass program for per-group sizes Gs."""
    f32 = mybir.dt.float32
    bf16 = mybir.dt.bfloat16
    fp8 = mybir.dt.float8e3
    nc = bacc.Bacc("TRN2", target_bir_lowering=False)

    geo = _geometry(S, A, Gs)
    K, CH, WF, FF = geo["K"], geo["CH"], geo["WF"], geo["FF"]
    SLP, NSL, CPT, T = geo["SLP"], geo["NSL"], geo["CPT"], geo["T"]
    NG, OW = geo["NG"], geo["OW"]
    assert S % 128 == 0 and A == 128

    wts = nc.dram_tensor("wts", [NG, 128, 2 * WF], fp8,
                         kind="ExternalInput").ap()
    acts = nc.dram_tensor("acts", [128, geo["AL"]], bf16,
                          kind="ExternalInput").ap()
    out = nc.dram_tensor("out", [geo["OL"]], bf16,
                         kind="ExternalOutput").ap()

    with tile.TileContext(nc) as tc:
        with (
            tc.tile_pool(name="a", bufs=1) as apool,
            tc.tile_pool(name="w", bufs=NG) as wpool,
            tc.tile_pool(name="o", bufs=1) as opool,
            tc.tile_pool(name="psum", bufs=8, space="PSUM") as psum,
        ):
            # all activations prefetched in one DMA (small: ~4KB/partition)
            at = apool.tile([128, geo["AL"]], bf16)
            nc.scalar.dma_start(at[:], acts[:])
            # shared output staging tile, flushed by consolidated DMAs
            os_t = opool.tile([128, NG * OW], bf16)

            for g in range(NG):
                G = Gs[g]
                ring = nc.sync if g % 2 == 0 else nc.scalar
                wt = wpool.tile([128, 2 * WF], fp8, tag="wt", name=f"wt_{g}")
                if g == 0:
                    # pipeline fill: one half per ring, in parallel
                    nc.sync.dma_start(wt[:, :WF], wts[0, :, :WF])
                    nc.scalar.dma_start(wt[:, WF:], wts[0, :, WF:])
                elif g == NG - 1:
                    # tail group: per-context-pair weight DMAs (alternating
                    # rings) so the last matmuls start as soon as their
                    # slice lands, completion receipts overlap, and the
                    # HWDGE completion-sem lanes aren't oversubscribed
                    for hf in range(2):
                        for cp in range(CH // 2):
                            lo = hf * WF + cp * 2 * K * S
                            r = nc.sync if (hf + cp) % 2 == 0 \
                                else nc.scalar
                            r.dma_start(wt[:, lo:lo + 2 * K * S],
                                        wts[g, :, lo:lo + 2 * K * S])
                else:
                    ring.dma_start(wt[:], wts[g])

                def views(c):
                    hf, ci = divmod(c, CH)
                    wv = wt[:, hf * WF + ci * K * S:
                            hf * WF + (ci + 1) * K * S] \
                        .rearrange("p (k s) -> p k s", k=K)
                    a0 = geo["offs_a"][g] + (hf * CH + ci) * K * G
                    av = at[:, a0:a0 + K * G] \
                        .rearrange("p (k g) -> p k g", k=K)
                    return wv, av

                ps_tiles = [psum.tile([128, FF * S], f32, tag="ps",
                                      name=f"ps_{g}_{t}")
                            for t in range(T)]
                # consecutive contexts cycle the NSL column tiles of the
                # PE array so their streams execute concurrently; each
                # context's K accumulating matmuls stay adjacent (the
                # LDWEIGHTS lookahead only tracks one shadow per tile)
                for c in range(CT):
                    wv, av = views(c)
                    t, r2 = divmod(c, CPT)
                    sl, cf = r2 % NSL, r2 // NSL
                    pslice = ps_tiles[t][sl * SLP:sl * SLP + G,
                                         cf * S:cf * S + S]
                    for k in range(K):
                        nc.tensor.matmul(
                            pslice,
                            lhsT=av[:, k, :],
                            rhs=wv[:, k, :],
                            start=(k == 0), stop=(k == K - 1),
                            tile_position=(0, sl * SLP))
                o0 = g * T * FF * S
                if g == NG - 1:
                    # tail group: split the drain across vector+scalar so
                    # the latency-critical final chain halves
                    for t in range(T):
                        half = FF * S // 2
                        nc.vector.tensor_copy(
                            os_t[:, o0 + t * FF * S:
                                 o0 + t * FF * S + half],
                            ps_tiles[t][:, :half])
                        nc.scalar.copy(
                            os_t[:, o0 + t * FF * S + half:
                                 o0 + (t + 1) * FF * S],
                            ps_tiles[t][:, half:])
                else:
                    for t in range(T):
                        eng = (nc.vector.tensor_copy if g % 2 == 0
                               else nc.scalar.copy)
                        eng(os_t[:, o0 + t * FF * S:
                                 o0 + (t + 1) * FF * S],
                            ps_tiles[t][:, :])

            # consolidated output flushes: one full-partition DMA per
            # split (garbage rows are cheaper than issue serialization
            # and HWDGE completion-sem lane pressure)
            for si, (a, b) in enumerate(geo["splits"]):
                dst = out[128 * a * OW: 128 * b * OW]
                ring = nc.sync if si % 2 == 0 else nc.scalar
                ring.dma_start(
                    dst.rearrange("(p w) -> p w", p=128),
                    os_t[:, a * OW:b * OW])

    nc.compile()
    return nc


def kernel(x, u, WA, WB, adj_xx, adj_xu, context, _trace=False):
    B, S = x.shape
    _, A = u.shape
    C = WA.shape[0]
    assert C % N_CORES == 0
    CP = C // N_CORES
    assert CP % CT == 0
    NG = CP // CT

    # ---- host-side shard: count-sorted contexts, dealt round-robin ----
    context = np.asarray(context)
    cnt = np.bincount(context, minlength=C)
    perm = np.argsort(-cnt, kind="stable")          # contexts by count desc
    # context at global rank r -> core r%8, position r//8; group = pos//CT.
    # All cores share one program, so G_g is set by the chunk's global max
    # count = count at rank g*CT*N_CORES.
    Gs = []
    for g in range(NG):
        m = int(cnt[perm[g * CT * N_CORES]])
        Gs.append(max(2, ((m + 1) // 2) * 2))

    geo = _geometry(S, A, Gs)
    HS, K, CH, WF = geo["HS"], geo["K"], geo["CH"], geo["WF"]
    FF, SLP, NSL, CPT, T, OW = (geo["FF"], geo["SLP"], geo["NSL"],
                                geo["CPT"], geo["T"], geo["OW"])

    order = np.argsort(context, kind="stable")
    starts = np.zeros(C + 1, np.int64)
    starts[1:] = np.cumsum(cnt)

    def group_rows(ctx_ids, G):
        """gidx [len,G] sample indices (clamped) + valid mask."""
        j = np.arange(G)
        cc = cnt[ctx_ids][:, None]
        valid = j[None, :] < cc
        pos = starts[ctx_ids][:, None] + np.minimum(j[None, :],
                                                    np.maximum(cc - 1, 0))
        return order[pos], valid

    inv = np.float32(1.0 / WSCALE)
    x = np.asarray(x, np.float32) * inv
    u = np.asarray(u, np.float32) * inv

    # pre-mask the weight banks, scale into e3m4 range, quantize on host
    Am = (np.asarray(WA, np.float32) * np.float32(WSCALE)
          * np.asarray(adj_xx, np.float32)).astype(FP8)    # [C, S, S]
    Bm = (np.asarray(WB, np.float32) * np.float32(WSCALE)
          * np.asarray(adj_xu, np.float32)).astype(FP8)    # [C, A, S]

    in_maps = []
    scatter = []   # per core: list of (ctx_ids, gidx, valid) per group
    for k in range(N_CORES):
        wblob = np.empty((NG, 128, 2, CH, K, S), FP8)
        ablob = np.zeros((128, geo["AL"]), BF16)
        sc = []
        for g in range(NG):
            G = Gs[g]
            ctx_ids = perm[(g * CT + np.arange(CT)) * N_CORES + k]
            gidx, valid = group_rows(ctx_ids, G)           # [CT, G]
            sc.append((ctx_ids, gidx, valid))
            XpT = x[gidx].transpose(0, 2, 1).astype(BF16)  # [CT, S, G]
            UpT = u[gidx].transpose(0, 2, 1).astype(BF16)  # [CT, A, G]
            wb = wblob[g].transpose(1, 2, 0, 3, 4)         # [2,CH,128,K,S]
            wb[..., 0, :] = Bm[ctx_ids].reshape(2, CH, 128, S)
            wb[..., 1:, :] = Am[ctx_ids].reshape(2, CH, HS, 128, S) \
                .transpose(0, 1, 3, 2, 4)
            A3 = ablob[:, geo["offs_a"][g]:
                       geo["offs_a"][g] + 2 * CH * K * G] \
                .reshape(128, 2, CH, K, G).transpose(1, 2, 3, 0, 4)
            A3[:, :, 0] = UpT.reshape(2, CH, 128, G)
            A3[:, :, 1:] = XpT.reshape(2, CH, HS, 128, G)
        in_maps.append({"wts": wblob.reshape(NG, 128, 2 * WF),
                        "acts": ablob})
        scatter.append(sc)

    if _trace:
        _install_profile_shim()
    nc = _build_program(S, A, Gs)
    res = run_bass_kernel_spmd(nc, in_maps, core_ids=list(range(N_CORES)),
                               trace=_trace)

    # unscatter: each split flush writes its own [128, (b-a)*OW] block
    out_full = np.zeros((B, S), np.float32)
    for k, r in enumerate(res.results):
        v = np.asarray(r["out"]).astype(np.float32)
        for a, b in geo["splits"]:
            blk = v[128 * a * OW: 128 * b * OW] \
                .reshape(128, b - a, T, FF, S)
            for g in range(a, b):
                ctx_ids, gidx, valid = scatter[k][g]
                for c in range(CT):
                    t, r2 = divmod(c, CPT)
                    sl, cf = r2 % NSL, r2 // NSL
                    rows = blk[sl * SLP:sl * SLP + Gs[g],
                               g - a, t, cf, :]              # [G, S]
                    m = valid[c]
                    out_full[gidx[c][m]] = rows[m]

    if _trace:
        return out_full, res

    return out_full


# revision 31
# speedup vs baseline: 1.1381x; 1.0506x over previous
"""Trainium2 Bass kernel for ContextHyperLinearSSM.

Computes out[b,:] = x[b,:] @ (WA[context[b]] * adj_xx) + u[b,:] @ (WB[context[b]] * adj_xu)

Strategy: shard the CONTEXT axis across the 8 cores (64 contexts each).
The host groups samples by context, masks the weight banks with the
adjacency masks, and quantizes the weights to fp8-e3m4 (x2^6 scale; the
inverse 2^-6 is folded into the bf16 activations — both scalings are exact
powers of two, so the only quantization error is the e3m4 weight rounding,
measured at 1.3e-2 absmax/scale against the fp32 reference).  Each core
streams its 64 contexts' weights from HBM exactly once and runs 3
accumulating mixed-dtype matmuls (bf16 stationary x fp8 moving) per
context.  Each sample's row is computed by exactly one core, so the
host-side unshard is a pure scatter.

Contexts are globally sorted by sample count and dealt round-robin to the
cores, so every core sees the same per-group padded size G_g (required:
one SPMD program serves all cores) and the padding tracks the count
distribution.  All activations ship in one prefetched DMA; weights stream
one merged DMA per group of CT contexts, groups alternating between the
two HWDGE rings (sync/scalar) so descriptor-generation cost and
completion-semaphore reuse never serialize the stream.  Group 0 is split
into per-half DMAs on both rings (shorter pipeline-fill latency) and the
final, smallest group into per-context DMAs (matmuls start as soon as
their slice of the last transfer lands).

Compute: all CT contexts of a group accumulate into ONE PSUM bank packed
as 4 x 32-aligned partition slots x 2 free halves; matmuls are emitted in
k-major waves cycling the four 128x32 column tiles of the PE array so
four matmuls execute concurrently.  One full-width copy per group (vector
and scalar engines alternating) drains PSUM into a shared bf16 staging
tile, which is flushed by a few consolidated multi-group DMAs.
"""

import sys

sys.path.insert(0, "/opt/trn_rl_repo")

import ml_dtypes
import numpy as np

import concourse.bass as bass
import concourse.mybir as mybir
import concourse.tile as tile
from concourse import bacc
from concourse.bass_utils import run_bass_kernel_spmd

N_CORES = 8
CT = 8  # contexts per PSUM group
WSCALE = 64.0  # 2^6: weights *= WSCALE (into e3m4 range), acts /= WSCALE

BF16 = ml_dtypes.bfloat16
FP8 = ml_dtypes.float8_e3m4


def _install_profile_shim():
    """Register the NTFF profile hook that trn_boot skips when
    antenv.axon_hooks is missing from the image (profiling only)."""
    import types
    if "antenv.axon_hooks" in sys.modules:
        return
    try:
        from trn_agent_boot.trn_boot import _ntff_profile_via_ctypes
        hook = _ntff_profile_via_ctypes("/opt/axon/libaxon_pjrt.so")
    except Exception:
        hook = None
    mod = types.ModuleType("antenv.axon_hooks")
    mod.get_axon_ntff_profile_hook = lambda: hook
    mod.set_axon_ntff_profile_hook = lambda h: None
    sys.modules["antenv.axon_hooks"] = mod


def _geometry(S, A, Gs):
    """Static geometry shared by host and device.

    PSUM packing: context c of a group -> bank t=c//CPT, partition slot
    sl=(c%CPT)%NSL (SLP-aligned), free half cf=(c%CPT)//NSL.
    """
    HS = S // 128
    K = HS + 1
    CH = CT // 2
    WF = CH * K * S
    FF = max(1, min(CT, 512 // S))
    SLP = 32 if max(Gs) <= 32 else 64
    NSL = 128 // SLP
    CPT = min(CT, NSL * FF)
    T = -(-CT // CPT)
    assert T * CPT == CT
    NG = len(Gs)
    # out-staging splits: [0,NG/2), [NG/2,NG-1), [NG-1,NG) — early flushes
    # plus a tiny final one (groups are sorted largest-first)
    splits = [(0, NG // 2), (NG // 2, NG - 1), (NG - 1, NG)]
    splits = [(a, b) for a, b in splits if b > a]
    off_a = 0
    offs_a = []
    for G in Gs:
        offs_a.append(off_a)
        off_a += 2 * CH * K * G
    OW = T * FF * S  # staging elems per partition line per group
    return dict(HS=HS, K=K, CH=CH, WF=WF, FF=FF, SLP=SLP, NSL=NSL,
                CPT=CPT, T=T, NG=NG, splits=splits, offs_a=offs_a,
                AL=off_a, OW=OW, OL=128 * NG * OW)


def _build_program(S, A, Gs):
    """Build the per-core Bass program for per-group sizes Gs."""
    f32 = mybir.dt.float32
    bf16 = mybir.dt.bfloat16
    fp8 = mybir.dt.float8e3
    nc = bacc.Bacc("TRN2", target_bir_lowering=False)

    geo = _geometry(S, A, Gs)
    K, CH, WF, FF = geo["K"], geo["CH"], geo["WF"], geo["FF"]
    SLP, NSL, CPT, T = geo["SLP"], geo["NSL"], geo["CPT"], geo["T"]
    NG, OW = geo["NG"], geo["OW"]
    assert S % 128 == 0 and A == 128

    wts = nc.dram_tensor("wts", [NG, 128, 2 * WF], fp8,
                         kind="ExternalInput").ap()
    acts = nc.dram_tensor("acts", [128, geo["AL"]], bf16,
                          kind="ExternalInput").ap()
    out = nc.dram_tensor("out", [geo["OL"]], bf16,
                         kind="ExternalOutput").ap()

    with tile.TileContext(nc) as tc:
        with (
            tc.tile_pool(name="a", bufs=1) as apool,
            tc.tile_pool(name="w", bufs=NG) as wpool,
            tc.tile_pool(name="o", bufs=1) as opool,
            tc.tile_pool(name="psum", bufs=8, space="PSUM") as psum,
        ):
            # all activations prefetched in one DMA (small: ~4KB/partition)
            at = apool.tile([128, geo["AL"]], bf16)
            nc.scalar.dma_start(at[:], acts[:])
            # shared output staging tile, flushed by consolidated DMAs
            os_t = opool.tile([128, NG * OW], bf16)

            for g in range(NG):
                G = Gs[g]
                ring = nc.sync if g % 2 == 0 else nc.scalar
                wt = wpool.tile([128, 2 * WF], fp8, tag="wt", name=f"wt_{g}")
                if g == 0:
                    # pipeline fill: one half per ring, in parallel
                    nc.sync.dma_start(wt[:, :WF], wts[0, :, :WF])
                    nc.scalar.dma_start(wt[:, WF:], wts[0, :, WF:])
                elif g == NG - 1:
                    # tail group: per-context weight DMAs (alternating
                    # rings) so the last matmuls start as soon as their
                    # slice lands and completion receipts overlap
                    for hf in range(2):
                        for ci in range(CH):
                            lo = hf * WF + ci * K * S
                            r = nc.sync if (hf * CH + ci) % 2 == 0 \
                                else nc.scalar
                            r.dma_start(wt[:, lo:lo + K * S],
                                        wts[g, :, lo:lo + K * S])
                else:
                    ring.dma_start(wt[:], wts[g])

                def views(c):
                    hf, ci = divmod(c, CH)
                    wv = wt[:, hf * WF + ci * K * S:
                            hf * WF + (ci + 1) * K * S] \
                        .rearrange("p (k s) -> p k s", k=K)
                    a0 = geo["offs_a"][g] + (hf * CH + ci) * K * G
                    av = at[:, a0:a0 + K * G] \
                        .rearrange("p (k g) -> p k g", k=K)
                    return wv, av

                ps_tiles = [psum.tile([128, FF * S], f32, tag="ps",
                                      name=f"ps_{g}_{t}")
                            for t in range(T)]
                # consecutive contexts cycle the NSL column tiles of the
                # PE array so their streams execute concurrently; each
                # context's K accumulating matmuls stay adjacent (the
                # LDWEIGHTS lookahead only tracks one shadow per tile)
                for c in range(CT):
                    wv, av = views(c)
                    t, r2 = divmod(c, CPT)
                    sl, cf = r2 % NSL, r2 // NSL
                    pslice = ps_tiles[t][sl * SLP:sl * SLP + G,
                                         cf * S:cf * S + S]
                    for k in range(K):
                        nc.tensor.matmul(
                            pslice,
                            lhsT=av[:, k, :],
                            rhs=wv[:, k, :],
                            start=(k == 0), stop=(k == K - 1),
                            tile_position=(0, sl * SLP))
                o0 = g * T * FF * S
                for t in range(T):
                    # one full-width drain per bank; engines alternate per
                    # group, but the tail group always uses the otherwise
                    # idle vector engine (scalar is busy issuing flushes)
                    eng = (nc.vector.tensor_copy
                           if (g % 2 == 0 or g == NG - 1)
                           else nc.scalar.copy)
                    eng(os_t[:, o0 + t * FF * S:
                             o0 + (t + 1) * FF * S],
                        ps_tiles[t][:, :])

            # consolidated output flushes: one full-partition DMA per
            # split (garbage rows are cheaper than issue serialization
            # and HWDGE completion-sem lane pressure)
            for si, (a, b) in enumerate(geo["splits"]):
                dst = out[128 * a * OW: 128 * b * OW]
                ring = nc.sync if si % 2 == 0 else nc.scalar
                ring.dma_start(
                    dst.rearrange("(p w) -> p w", p=128),
                    os_t[:, a * OW:b * OW])

    nc.compile()
    return nc


def kernel(x, u, WA, WB, adj_xx, adj_xu, context, _trace=False):
    B, S = x.shape
    _, A = u.shape
    C = WA.shape[0]
    assert C % N_CORES == 0
    CP = C // N_CORES
    assert CP % CT == 0
    NG = CP // CT

    # ---- host-side shard: count-sorted contexts, dealt round-robin ----
    context = np.asarray(context)
    cnt = np.bincount(context, minlength=C)
    perm = np.argsort(-cnt, kind="stable")          # contexts by count desc
    # context at global rank r -> core r%8, position r//8; group = pos//CT.
    # All cores share one program, so G_g is set by the chunk's global max
    # count = count at rank g*CT*N_CORES.
    Gs = []
    for g in range(NG):
        m = int(cnt[perm[g * CT * N_CORES]])
        Gs.append(max(2, ((m + 1) // 2) * 2))

    geo = _geometry(S, A, Gs)
    HS, K, CH, WF = geo["HS"], geo["K"], geo["CH"], geo["WF"]
    FF, SLP, NSL, CPT, T, OW = (geo["FF"], geo["SLP"], geo["NSL"],
                                geo["CPT"], geo["T"], geo["OW"])

    order = np.argsort(context, kind="stable")
    starts = np.zeros(C + 1, np.int64)
    starts[1:] = np.cumsum(cnt)

    def group_rows(ctx_ids, G):
        """gidx [len,G] sample indices (clamped) + valid mask."""
        j = np.arange(G)
        cc = cnt[ctx_ids][:, None]
        valid = j[None, :] < cc
        pos = starts[ctx_ids][:, None] + np.minimum(j[None, :],
                                                    np.maximum(cc - 1, 0))
        return order[pos], valid

    inv = np.float32(1.0 / WSCALE)
    x = np.asarray(x, np.float32) * inv
    u = np.asarray(u, np.float32) * inv

    # pre-mask the weight banks, scale into e3m4 range, quantize on host
    Am = (np.asarray(WA, np.float32) * np.float32(WSCALE)
          * np.asarray(adj_xx, np.float32)).astype(FP8)    # [C, S, S]
    Bm = (np.asarray(WB, np.float32) * np.float32(WSCALE)
          * np.asarray(adj_xu, np.float32)).astype(FP8)    # [C, A, S]

    in_maps = []
    scatter = []   # per core: list of (ctx_ids, gidx, valid) per group
    for k in range(N_CORES):
        wblob = np.empty((NG, 128, 2, CH, K, S), FP8)
        ablob = np.zeros((128, geo["AL"]), BF16)
        sc = []
        for g in range(NG):
            G = Gs[g]
            ctx_ids = perm[(g * CT + np.arange(CT)) * N_CORES + k]
            gidx, valid = group_rows(ctx_ids, G)           # [CT, G]
            sc.append((ctx_ids, gidx, valid))
            XpT = x[gidx].transpose(0, 2, 1).astype(BF16)  # [CT, S, G]
            UpT = u[gidx].transpose(0, 2, 1).astype(BF16)  # [CT, A, G]
            wb = wblob[g].transpose(1, 2, 0, 3, 4)         # [2,CH,128,K,S]
            wb[..., 0, :] = Bm[ctx_ids].reshape(2, CH, 128, S)
            wb[..., 1:, :] = Am[ctx_ids].reshape(2, CH, HS, 128, S) \
                .transpose(0, 1, 3, 2, 4)
            A3 = ablob[:, geo["offs_a"][g]:
                       geo["offs_a"][g] + 2 * CH * K * G] \
                .reshape(128, 2, CH, K, G).transpose(1, 2, 3, 0, 4)
            A3[:, :, 0] = UpT.reshape(2, CH, 128, G)
            A3[:, :, 1:] = XpT.reshape(2, CH, HS, 128, G)
        in_maps.append({"wts": wblob.reshape(NG, 128, 2 * WF),
                        "acts": ablob})
        scatter.append(sc)

    if _trace:
        _install_profile_shim()
    nc = _build_program(S, A, Gs)
    res = run_bass_kernel_spmd(nc, in_maps, core_ids=list(range(N_CORES)),
                               trace=_trace)

    # unscatter: each split flush writes its own [128, (b-a)*OW] block
    out_full = np.zeros((B, S), np.float32)
    for k, r in enumerate(res.results):
        v = np.asarray(r["out"]).astype(np.float32)
        for a, b in geo["splits"]:
            blk = v[128 * a * OW: 128 * b * OW] \
                .reshape(128, b - a, T, FF, S)
            for g in range(a, b):
                ctx_ids, gidx, valid = scatter[k][g]
                for c in range(CT):
                    t, r2 = divmod(c, CPT)
                    sl, cf = r2 % NSL, r2 // NSL
                    rows = blk[sl * SLP:sl * SLP + Gs[g],
                               g - a, t, cf, :]              # [G, S]
                    m = valid[c]
                    out_full[gidx[c][m]] = rows[m]

    if _trace:
        return out_full, res

    return out_full
